# revision 10
# baseline (speedup 1.0000x reference)
"""Multi-head attention (B=4, L=2048, D=512, H=8) on 8 Trainium2 NeuronCores.

Sharding: core = (batch b, head-group hg) -> each core handles 1 batch and 4
heads (tensor-parallel column-shard of Wq/Wk/Wv, row-shard of Wo). The two
head-group partial outputs per batch are summed on the host (the TP
all-reduce step of the gather).

Device dataflow (all f32, everything contraction-on-partitions, zero on-chip
transposes):
  - Host pre-transposes activations (qT/kT/vT: [512, 2048]) and weights.
  - Projections: qhT/khT = Wx^T.T @ xT -> [64*4, 2048] per-head-transposed
    layouts; vh = xT.T @ WvT -> [2048, 4*65] (keys on partitions) with a
    65th "ones" column per head.
  - Mask folding: vh rows of masked keys are zeroed (masked keys then
    contribute nothing to either the context numerator or - via the ones
    column - the softmax denominator). The ones column is overwritten with
    the 0/1 mask, so column 64 of the second matmul output IS the softmax
    denominator sum.
  - Scores are computed transposed: ST[k, q] = khT_h.T @ qhT_h, exp on
    ScalarE straight out of PSUM (no max-subtraction: scores ~ N(0,1), the
    shift is mathematically redundant), then ctxT[dk+1, q] accumulates
    vh_h.T @ exp(ST) over key chunks.
  - Normalization once on the small ctxT: r = 1/sums broadcast across
    partitions with a rank-1 PE matmul, one elementwise multiply.
  - Output projection O = ctxT.T @ WoT per 128-row chunk, DMA to DRAM.
"""
import os
import sys

import numpy as np

for _p in ("/opt/trn_rl_repo", "/root/.axon_site/_ro/trn_rl_repo"):
    if os.path.isdir(_p) and _p not in sys.path:
        sys.path.insert(0, _p)

B, L, D, H = 4, 2048, 512, 8
DK = D // H          # 64
HPG = 4              # heads per group
GD = HPG * DK        # 256
P = 128
NKC = L // P         # 16 key chunks
NLB = L // 512       # 4 l-blocks of 512
NLC = L // P         # 16 l chunks

_CACHE: dict = {}
# test harness hooks: set _RUN_OPTS["trace"]=True to request an NTFF profile;
# the last BassKernelResults lands in _CACHE["last_result"].
_RUN_OPTS: dict = {"trace": False}


def _emit_ctx(nc, prev, ctxp, vh_sb, hp):
    """Second attention matmul for one key chunk (both heads of the pair)."""
    ptp, kc = prev
    for hi in range(2):
        vlhsT = vh_sb[:, kc, 2 * hp + hi, :]
        for j in range(2):
            nc.tensor.matmul(
                ctxp[hi][:, j * 512:(j + 1) * 512],
                lhsT=vlhsT,
                rhs=ptp[hi][:, j * 512:(j + 1) * 512],
                start=(kc == 0), stop=(kc == NKC - 1),
            )


def _build_nc(ndc: int):
    """Build + compile the Bass program. ndc=4 normally, 5 when q/k/v biases
    are nonzero (extra contraction chunk carrying a ones row x bias row)."""
    from contextlib import ExitStack

    import concourse.bacc as bacc
    import concourse.tile as tile
    from concourse import mybir

    f32 = mybir.dt.float32
    f32r = mybir.dt.float32r
    EXP = mybir.ActivationFunctionType.Exp
    MULT = mybir.AluOpType.mult

    nc = bacc.Bacc("TRN2", target_bir_lowering=False, debug=False, num_devices=8)

    qT = nc.dram_tensor("qT", [ndc, P, L], f32r, kind="ExternalInput").ap()
    kT = nc.dram_tensor("kT", [ndc, P, L], f32r, kind="ExternalInput").ap()
    vT = nc.dram_tensor("vT", [ndc, P, L], f32r, kind="ExternalInput").ap()
    wqT = nc.dram_tensor("wqT", [P, ndc, GD], f32r, kind="ExternalInput").ap()
    wkT = nc.dram_tensor("wkT", [P, ndc, GD], f32r, kind="ExternalInput").ap()
    wvT = nc.dram_tensor("wvT", [P, ndc, HPG * 65], f32r, kind="ExternalInput").ap()
    woT = nc.dram_tensor("woT", [P, 2, D], f32r, kind="ExternalInput").ap()
    maskp = nc.dram_tensor("maskp", [P, NKC], f32, kind="ExternalInput").ap()
    o = nc.dram_tensor("o", [NLC, P, D], f32, kind="ExternalOutput").ap()

    with ExitStack() as ctx:
        tc = ctx.enter_context(tile.TileContext(nc))
        const = ctx.enter_context(tc.tile_pool(name="const", bufs=1))
        persist = ctx.enter_context(tc.tile_pool(name="persist", bufs=1))

        # constants / weights
        wq_sb = const.tile([P, ndc, GD], f32r)
        wk_sb = const.tile([P, ndc, GD], f32r)
        wv_sb = const.tile([P, ndc, HPG * 65], f32r)
        wo_sb = const.tile([P, 2, D], f32r)
        maskp_sb = const.tile([P, NKC], f32)
        ones_sb = const.tile([1, DK], f32)
        dummy_sb = const.tile([1, 8], f32)
        nc.sync.dma_start(wq_sb, wqT)
        nc.sync.dma_start(wk_sb, wkT)
        nc.sync.dma_start(wv_sb, wvT)
        nc.sync.dma_start(wo_sb, woT)
        nc.sync.dma_start(maskp_sb, maskp)
        nc.vector.memset(ones_sb, 1.0)
        # preload the exp table set early (overlaps the projection phase)
        nc.vector.memset(dummy_sb, 0.0)
        nc.scalar.activation(dummy_sb, dummy_sb, EXP)

        # persistent activations
        qh_sb = [persist.tile([P, L], f32r, name=f"qh{i}") for i in range(2)]
        kh_sb = [persist.tile([P, L], f32r, name=f"kh{i}") for i in range(2)]
        vh_sb = persist.tile([P, NKC, HPG, 65], f32r, name="vh")
        ctx_sb = [persist.tile([P, L], f32r, name=f"ctx{i}") for i in range(2)]

        # ---------------- projections ----------------
        with tc.tile_pool(name="xT", bufs=ndc) as xpool, \
             tc.tile_pool(name="ppsum", bufs=4, space="PSUM") as ppsum:
            for xap, w_sb, dst in ((kT, wk_sb, kh_sb), (qT, wq_sb, qh_sb)):
                xt = [xpool.tile([P, L], f32r, tag="xT", name=f"xt{dc}") for dc in range(ndc)]
                for dc in range(ndc):
                    nc.sync.dma_start(xt[dc], xap[dc])
                for hp in range(2):
                    for lb in range(NLB):
                        ps = ppsum.tile([P, 512], f32, tag="pp", name="ps_qk")
                        for dc in range(ndc):
                            nc.tensor.matmul(
                                ps,
                                lhsT=w_sb[:, dc, hp * P:(hp + 1) * P],
                                rhs=xt[dc][:, lb * 512:(lb + 1) * 512],
                                start=(dc == 0),
                                stop=(dc == ndc - 1),
                            )
                        nc.vector.tensor_copy(dst[hp][:, lb * 512:(lb + 1) * 512], ps)
            # V projection: vh[l, :] with mask fold
            xt = [xpool.tile([P, L], f32r, tag="xT", name=f"xt{dc}") for dc in range(ndc)]
            for dc in range(ndc):
                nc.sync.dma_start(xt[dc], vT[dc])
            for lc in range(NLC):
                ps = ppsum.tile([P, HPG * 65], f32, tag="pp", name="ps_v")
                for dc in range(ndc):
                    nc.tensor.matmul(
                        ps,
                        lhsT=xt[dc][:, lc * P:(lc + 1) * P],
                        rhs=wv_sb[:, dc, :],
                        start=(dc == 0),
                        stop=(dc == ndc - 1),
                    )
                nc.vector.tensor_scalar_mul(
                    vh_sb[:, lc, :, :], ps.rearrange("p (h d) -> p h d", h=HPG),
                    maskp_sb[:, lc:lc + 1],
                )
                # ones-column -> 0/1 mask column (weights there are zero)
                nc.vector.tensor_copy(
                    vh_sb[:, lc, :, DK:DK + 1],
                    maskp_sb[:, lc:lc + 1, None].to_broadcast((P, HPG, 1)),
                )

        # ---------------- attention + output projection ----------------
        with tc.tile_pool(name="spsum", bufs=2, space="PSUM") as s_pool, \
             tc.tile_pool(name="cpsum", bufs=2, space="PSUM") as ctx_pool, \
             tc.tile_pool(name="pt", bufs=4) as pt_pool, \
             tc.tile_pool(name="nrm", bufs=2) as nrm_pool, \
             tc.tile_pool(name="osb", bufs=3) as o_pool:
            for q2 in range(2):           # q halves of 1024
                q0 = q2 * 1024
                for hp in range(2):       # head pairs, row-tiled concurrently
                    ctxp = [ctx_pool.tile([65, 1024], f32, tag="ctx",
                                          name=f"ctx{hi}") for hi in range(2)]
                    prev = None
                    for kc in range(NKC):
                        # scores for both heads; hi=1 runs on array rows
                        # 64-127 concurrently with hi=0 (tile_position auto)
                        spair = [s_pool.tile([P, 1024], f32, tag="s",
                                             name=f"s{hi}") for hi in range(2)]
                        for hi in range(2):
                            lhsT = kh_sb[hp][hi * DK:(hi + 1) * DK,
                                             kc * P:(kc + 1) * P]
                            for j in range(2):
                                nc.tensor.matmul(
                                    spair[hi][:, j * 512:(j + 1) * 512],
                                    lhsT=lhsT,
                                    rhs=qh_sb[hp][hi * DK:(hi + 1) * DK,
                                                  q0 + j * 512:q0 + (j + 1) * 512],
                                    start=True, stop=True,
                                )
                        if prev is not None:
                            _emit_ctx(nc, prev, ctxp, vh_sb, hp)
                        ptp = [pt_pool.tile([P, 1024], f32r, tag="pt",
                                            name=f"pt{hi}") for hi in range(2)]
                        for hi in range(2):
                            nc.scalar.activation(ptp[hi], spair[hi], EXP)
                        prev = (ptp, kc)
                    _emit_ctx(nc, prev, ctxp, vh_sb, hp)
                    # normalize: ctx_sb = ctx_ps[0:64] * (1/sums) broadcast
                    for hi in range(2):
                        hb = hi * DK
                        srow = nrm_pool.tile([1, 1024], f32, tag="srow", name="srow")
                        nc.vector.tensor_copy(srow, ctxp[hi][64:65, :])
                        rrow = nrm_pool.tile([1, 1024], f32, tag="rrow", name="rrow")
                        nc.vector.reciprocal_approx_fast(rrow, srow)
                        bc_sb = nrm_pool.tile([DK, 1024], f32, tag="bc", name="bc_sb")
                        nc.gpsimd.partition_broadcast(bc_sb, rrow)
                        nc.vector.tensor_tensor(
                            ctx_sb[hp][hb:hb + DK, q0:q0 + 1024],
                            ctxp[hi][0:DK, :],
                            bc_sb,
                            MULT,
                        )
                # output projection for this q half (8 l-chunks of 128)
                for lc in range(q2 * 8, q2 * 8 + 8):
                    ps = ctx_pool.tile([P, 512], f32, tag="ctx", name="ps_o")
                    for c2 in range(2):
                        nc.tensor.matmul(
                            ps,
                            lhsT=ctx_sb[c2][:, lc * P:(lc + 1) * P],
                            rhs=wo_sb[:, c2, :],
                            start=(c2 == 0), stop=(c2 == 1),
                        )
                    ot = o_pool.tile([P, D], f32, tag="o", name="ot")
                    nc.vector.tensor_copy(ot, ps)
                    nc.sync.dma_start(o[lc], ot)

    nc.compile()
    return nc


def _get_nc(ndc: int):
    key = ("nc", ndc)
    if key not in _CACHE:
        _CACHE[key] = _build_nc(ndc)
    return _CACHE[key]


def _prep_core(core, q, k, v, masks, wq_w, wq_b, wk_w, wk_b, wv_w, wv_b, ndc):
    b, hg = core // 2, core % 2
    rows = slice(hg * GD, (hg + 1) * GD)
    scale = np.float32(1.0 / np.sqrt(DK))

    def xt_chunks(x):
        xt = np.zeros((ndc, P, L), np.float32)
        xt[:4] = np.ascontiguousarray(x.T).reshape(4, P, L)
        if ndc == 5:
            xt[4, 0, :] = 1.0  # ones row for the bias chunk
        return xt

    def w_chunks(wT, bias, width):
        w = np.zeros((ndc * P, width), np.float32)
        w[:D] = wT
        if ndc == 5:
            w[D] = bias
        return np.ascontiguousarray(w.reshape(ndc, P, width).transpose(1, 0, 2))

    wqT = (wq_w[rows, :].T * scale).astype(np.float32)          # [512, 256]
    wkT = wk_w[rows, :].T.astype(np.float32)
    wvT = np.zeros((D, HPG * 65), np.float32)
    wvb = np.zeros((HPG * 65,), np.float32)
    wvg = wv_w[rows, :]
    for hh in range(HPG):
        wvT[:, hh * 65:hh * 65 + DK] = wvg[hh * DK:(hh + 1) * DK].T
        wvb[hh * 65:hh * 65 + DK] = wv_b[rows][hh * DK:(hh + 1) * DK]
    return {
        "qT": xt_chunks(q[b]),
        "kT": xt_chunks(k[b]),
        "vT": xt_chunks(v[b]),
        "wqT": w_chunks(wqT, wq_b[rows] * scale, GD),
        "wkT": w_chunks(wkT, wk_b[rows], GD),
        "wvT": w_chunks(wvT, wvb, HPG * 65),
        "maskp": np.ascontiguousarray(
            masks[b].reshape(NKC, P).T.astype(np.float32)),
    }


def kernel(q, k, v, masks, wq_w, wq_b, wk_w, wk_b, wv_w, wv_b, wo_w, wo_b):
    from concourse.bass_utils import run_bass_kernel_spmd

    q = np.asarray(q, np.float32)
    k = np.asarray(k, np.float32)
    v = np.asarray(v, np.float32)
    masks_np = np.asarray(masks)
    args = [np.asarray(a, np.float32) for a in
            (wq_w, wq_b, wk_w, wk_b, wv_w, wv_b, wo_w, wo_b)]
    wq_w, wq_b, wk_w, wk_b, wv_w, wv_b, wo_w, wo_b = args

    ndc = 5 if (np.any(wq_b) or np.any(wk_b) or np.any(wv_b)) else 4
    nc = _get_nc(ndc)

    in_maps = []
    for core in range(8):
        m = _prep_core(core, q, k, v, masks_np, wq_w, wq_b, wk_w, wk_b,
                       wv_w, wv_b, ndc)
        hg = core % 2
        rows = slice(hg * GD, (hg + 1) * GD)
        m["woT"] = np.ascontiguousarray(
            wo_w[:, rows].T.reshape(2, P, D).transpose(1, 0, 2))
        in_maps.append(m)

    res = run_bass_kernel_spmd(nc, in_maps, core_ids=list(range(8)),
                               trace=_RUN_OPTS.get("trace", False),
                               tmpdir=_RUN_OPTS.get("tmpdir"))
    _CACHE["last_result"] = res
    outs = res.results

    O = np.zeros((B, L, D), np.float32)
    for b in range(B):
        O[b] = (outs[2 * b]["o"].reshape(L, D)
                + outs[2 * b + 1]["o"].reshape(L, D))
    O += (wv_b @ wo_w.T + wo_b)[None, None, :] if ndc == 4 else wo_b[None, None, :]
    return O


# revision 13
# speedup vs baseline: 1.6977x; 1.6977x over previous
"""Multi-head attention (B=4, L=2048, D=512, H=8) on 8 Trainium2 NeuronCores.

Sharding: core = (batch b, head-group hg) -> each core handles 1 batch and 4
heads (tensor-parallel column-shard of Wq/Wk/Wv, row-shard of Wo). The two
head-group partial outputs per batch are summed on the host (the TP
all-reduce step of the gather).

Device dataflow (all f32, everything contraction-on-partitions, zero on-chip
transposes):
  - Host pre-transposes activations (qT/kT/vT: [512, 2048]) and weights.
  - Projections: qhT/khT = Wx^T.T @ xT -> [64*4, 2048] per-head-transposed
    layouts; vh = xT.T @ WvT -> [2048, 4*65] (keys on partitions) with a
    65th "ones" column per head.
  - Mask folding: vh rows of masked keys are zeroed (masked keys then
    contribute nothing to either the context numerator or - via the ones
    column - the softmax denominator). The ones column is overwritten with
    the 0/1 mask, so column 64 of the second matmul output IS the softmax
    denominator sum.
  - Scores are computed transposed: ST[k, q] = khT_h.T @ qhT_h, exp on
    ScalarE straight out of PSUM (no max-subtraction: scores ~ N(0,1), the
    shift is mathematically redundant), then ctxT[dk+1, q] accumulates
    vh_h.T @ exp(ST) over key chunks.
  - Normalization once on the small ctxT: r = 1/sums broadcast across
    partitions with a rank-1 PE matmul, one elementwise multiply.
  - Output projection O = ctxT.T @ WoT per 128-row chunk, DMA to DRAM.
"""
import os
import sys

import numpy as np

for _p in ("/opt/trn_rl_repo", "/root/.axon_site/_ro/trn_rl_repo"):
    if os.path.isdir(_p) and _p not in sys.path:
        sys.path.insert(0, _p)

B, L, D, H = 4, 2048, 512, 8
DK = D // H          # 64
HPG = 4              # heads per group
GD = HPG * DK        # 256
P = 128
NKC = L // P         # 16 key chunks
NLB = L // 512       # 4 l-blocks of 512
NLC = L // P         # 16 l chunks

_CACHE: dict = {}
# test harness hooks: set _RUN_OPTS["trace"]=True to request an NTFF profile;
# the last BassKernelResults lands in _CACHE["last_result"].
_RUN_OPTS: dict = {"trace": False}


def _build_nc(ndc: int, nkc: int):
    """Build + compile the Bass program.

    ndc: 4 normally, 5 when q/k/v biases are nonzero (extra contraction chunk
    carrying a ones row x bias row).
    nkc: number of 128-key chunks after host-side compaction of masked-out
    keys (masked keys contribute exactly nothing - their vh rows and mask
    column are zero - so only unmasked keys are shipped/computed; padding
    keys have zero vh/mask rows and zero kh, so exp(0)=1 times 0 = 0).
    """
    from contextlib import ExitStack

    import concourse.bacc as bacc
    import concourse.tile as tile
    from concourse import mybir

    f32 = mybir.dt.float32
    f32r = mybir.dt.float32r
    EXP = mybir.ActivationFunctionType.Exp
    MULT = mybir.AluOpType.mult

    nc = bacc.Bacc("TRN2", target_bir_lowering=False, debug=False, num_devices=8)

    NKP = nkc * P
    NKB = (NKP + 511) // 512  # key-axis 512-blocks for the K projection
    qT = nc.dram_tensor("qT", [ndc, P, L], f32r, kind="ExternalInput").ap()
    kT = nc.dram_tensor("kT", [ndc, P, NKP], f32r, kind="ExternalInput").ap()
    vT = nc.dram_tensor("vT", [ndc, P, NKP], f32r, kind="ExternalInput").ap()
    wqT = nc.dram_tensor("wqT", [P, ndc, GD], f32r, kind="ExternalInput").ap()
    wkT = nc.dram_tensor("wkT", [P, ndc, GD], f32r, kind="ExternalInput").ap()
    wvT = nc.dram_tensor("wvT", [P, ndc, HPG * 65], f32r, kind="ExternalInput").ap()
    woT = nc.dram_tensor("woT", [P, 2, D], f32r, kind="ExternalInput").ap()
    maskp = nc.dram_tensor("maskp", [P, nkc], f32, kind="ExternalInput").ap()
    o = nc.dram_tensor("o", [NLC, P, D], f32, kind="ExternalOutput").ap()

    with ExitStack() as ctx:
        tc = ctx.enter_context(tile.TileContext(nc))
        const = ctx.enter_context(tc.tile_pool(name="const", bufs=1))
        persist = ctx.enter_context(tc.tile_pool(name="persist", bufs=1))

        # constants / weights
        wq_sb = const.tile([P, ndc, GD], f32r)
        wk_sb = const.tile([P, ndc, GD], f32r)
        wv_sb = const.tile([P, ndc, HPG * 65], f32r)
        wo_sb = const.tile([P, 2, D], f32r)
        maskp_sb = const.tile([P, nkc], f32)
        ones_sb = const.tile([1, DK], f32)
        dummy_sb = const.tile([1, 8], f32)
        nc.sync.dma_start(wq_sb, wqT)
        nc.sync.dma_start(wk_sb, wkT)
        nc.sync.dma_start(wv_sb, wvT)
        nc.sync.dma_start(wo_sb, woT)
        nc.sync.dma_start(maskp_sb, maskp)
        nc.vector.memset(ones_sb, 1.0)
        # preload the exp table set early (overlaps the projection phase)
        nc.vector.memset(dummy_sb, 0.0)
        nc.scalar.activation(dummy_sb, dummy_sb, EXP)

        # persistent activations
        qh_sb = [persist.tile([P, L], f32r, name=f"qh{i}") for i in range(2)]
        kh_sb = [persist.tile([P, NKP], f32r, name=f"kh{i}") for i in range(2)]
        vh_sb = persist.tile([P, nkc, HPG, 65], f32r, name="vh")
        ctx_sb = [persist.tile([P, L], f32r, name=f"ctx{i}") for i in range(2)]

        # ---------------- projections ----------------
        with tc.tile_pool(name="xT", bufs=ndc) as xpool, \
             tc.tile_pool(name="ppsum", bufs=4, space="PSUM") as ppsum:
            for xap, w_sb, dst, xw in ((kT, wk_sb, kh_sb, NKP),
                                       (qT, wq_sb, qh_sb, L)):
                xt = [xpool.tile([P, L], f32r, tag="xT", name=f"xt{dc}") for dc in range(ndc)]
                for dc in range(ndc):
                    nc.sync.dma_start(xt[dc][:, 0:xw], xap[dc])
                for hp in range(2):
                    for lb in range((xw + 511) // 512):
                        nb = min(512, xw - lb * 512)
                        ps = ppsum.tile([P, 512], f32, tag="pp", name="ps_qk")
                        for dc in range(ndc):
                            nc.tensor.matmul(
                                ps[:, 0:nb],
                                lhsT=w_sb[:, dc, hp * P:(hp + 1) * P],
                                rhs=xt[dc][:, lb * 512:lb * 512 + nb],
                                start=(dc == 0),
                                stop=(dc == ndc - 1),
                            )
                        nc.vector.tensor_copy(dst[hp][:, lb * 512:lb * 512 + nb],
                                              ps[:, 0:nb])
            # V projection: vh[l, :] with mask fold
            xt = [xpool.tile([P, L], f32r, tag="xT", name=f"xt{dc}") for dc in range(ndc)]
            for dc in range(ndc):
                nc.sync.dma_start(xt[dc][:, 0:NKP], vT[dc])
            for lc in range(nkc):
                ps = ppsum.tile([P, HPG * 65], f32, tag="pp", name="ps_v")
                for dc in range(ndc):
                    nc.tensor.matmul(
                        ps,
                        lhsT=xt[dc][:, lc * P:(lc + 1) * P],
                        rhs=wv_sb[:, dc, :],
                        start=(dc == 0),
                        stop=(dc == ndc - 1),
                    )
                nc.vector.tensor_scalar_mul(
                    vh_sb[:, lc, :, :], ps.rearrange("p (h d) -> p h d", h=HPG),
                    maskp_sb[:, lc:lc + 1],
                )
                # ones-column -> 0/1 mask column (weights there are zero)
                nc.vector.tensor_copy(
                    vh_sb[:, lc, :, DK:DK + 1],
                    maskp_sb[:, lc:lc + 1, None].to_broadcast((P, HPG, 1)),
                )

        # ---------------- attention + output projection ----------------
        with tc.tile_pool(name="spsum", bufs=3, space="PSUM") as s_pool, \
             tc.tile_pool(name="cpsum", bufs=1, space="PSUM") as ctx_pool, \
             tc.tile_pool(name="pt", bufs=4) as pt_pool, \
             tc.tile_pool(name="nrm", bufs=2) as nrm_pool, \
             tc.tile_pool(name="osb", bufs=3) as o_pool:
            for q2 in range(2):           # q halves of 1024
                q0 = q2 * 1024
                for h in range(HPG):
                    hp, hb = h // 2, (h % 2) * DK
                    ctx_ps = ctx_pool.tile([65, 1024], f32, tag="ctx", name="ctx_ps")
                    for kc in range(nkc):
                        s_ps = s_pool.tile([P, 1024], f32, tag="s", name="s_ps")
                        lhsT = kh_sb[hp][hb:hb + DK, kc * P:(kc + 1) * P]
                        for j in range(2):
                            nc.tensor.matmul(
                                s_ps[:, j * 512:(j + 1) * 512],
                                lhsT=lhsT,
                                rhs=qh_sb[hp][hb:hb + DK, q0 + j * 512:q0 + (j + 1) * 512],
                                start=True, stop=True,
                            )
                        pt = pt_pool.tile([P, 1024], f32r, tag="pt", name="pt")
                        nc.scalar.activation(pt, s_ps, EXP)
                        vlhsT = vh_sb[:, kc, h, :]
                        for j in range(2):
                            nc.tensor.matmul(
                                ctx_ps[:, j * 512:(j + 1) * 512],
                                lhsT=vlhsT,
                                rhs=pt[:, j * 512:(j + 1) * 512],
                                start=(kc == 0), stop=(kc == nkc - 1),
                            )
                    # normalize: ctx_sb = ctx_ps[0:64] * (1/sums) broadcast
                    srow = nrm_pool.tile([1, 1024], f32, tag="srow", name="srow")
                    nc.vector.tensor_copy(srow, ctx_ps[64:65, :])
                    rrow = nrm_pool.tile([1, 1024], f32, tag="rrow", name="rrow")
                    nc.vector.reciprocal_approx_fast(rrow, srow)
                    bc_sb = nrm_pool.tile([DK, 1024], f32, tag="bc", name="bc_sb")
                    nc.gpsimd.partition_broadcast(bc_sb, rrow)
                    nc.vector.tensor_tensor(
                        ctx_sb[hp][hb:hb + DK, q0:q0 + 1024],
                        ctx_ps[0:DK, :],
                        bc_sb,
                        MULT,
                    )
                # output projection for this q half (8 l-chunks of 128)
                for lc in range(q2 * 8, q2 * 8 + 8):
                    ps = s_pool.tile([P, 1024], f32, tag="s", name="ps_o")
                    for c2 in range(2):
                        nc.tensor.matmul(
                            ps[:, 0:D],
                            lhsT=ctx_sb[c2][:, lc * P:(lc + 1) * P],
                            rhs=wo_sb[:, c2, :],
                            start=(c2 == 0), stop=(c2 == 1),
                        )
                    ot = o_pool.tile([P, D], f32, tag="o", name="ot")
                    nc.scalar.copy(ot, ps[:, 0:D])
                    nc.sync.dma_start(o[lc], ot)

    nc.compile()
    return nc


def _get_nc(ndc: int, nkc: int):
    key = ("nc", ndc, nkc)
    if key not in _CACHE:
        _CACHE[key] = _build_nc(ndc, nkc)
    return _CACHE[key]


def _prep_core(core, q, k, v, masks, wq_w, wq_b, wk_w, wk_b, wv_w, wv_b, ndc,
               nkc):
    b, hg = core // 2, core % 2
    rows = slice(hg * GD, (hg + 1) * GD)
    scale = np.float32(1.0 / np.sqrt(DK))
    NKP = nkc * P
    idx = np.nonzero(masks[b])[0]          # unmasked key positions

    def xt_chunks(x, compact):
        w = NKP if compact else L
        xt = np.zeros((ndc, P, w), np.float32)
        xs = x[idx] if compact else x      # [nk or L, 512]
        xt[:4, :, :xs.shape[0]] = np.ascontiguousarray(xs.T).reshape(4, P, -1)
        if ndc == 5:
            xt[4, 0, :xs.shape[0]] = 1.0   # ones row for the bias chunk
        return xt

    def w_chunks(wT, bias, width):
        w = np.zeros((ndc * P, width), np.float32)
        w[:D] = wT
        if ndc == 5:
            w[D] = bias
        return np.ascontiguousarray(w.reshape(ndc, P, width).transpose(1, 0, 2))

    wqT = (wq_w[rows, :].T * scale).astype(np.float32)          # [512, 256]
    wkT = wk_w[rows, :].T.astype(np.float32)
    wvT = np.zeros((D, HPG * 65), np.float32)
    wvb = np.zeros((HPG * 65,), np.float32)
    wvg = wv_w[rows, :]
    for hh in range(HPG):
        wvT[:, hh * 65:hh * 65 + DK] = wvg[hh * DK:(hh + 1) * DK].T
        wvb[hh * 65:hh * 65 + DK] = wv_b[rows][hh * DK:(hh + 1) * DK]
    maskc = np.zeros((NKP,), np.float32)
    maskc[:len(idx)] = 1.0
    return {
        "qT": xt_chunks(q[b], False),
        "kT": xt_chunks(k[b], True),
        "vT": xt_chunks(v[b], True),
        "wqT": w_chunks(wqT, wq_b[rows] * scale, GD),
        "wkT": w_chunks(wkT, wk_b[rows], GD),
        "wvT": w_chunks(wvT, wvb, HPG * 65),
        "maskp": np.ascontiguousarray(
            maskc.reshape(nkc, P).T.astype(np.float32)),
    }


def kernel(q, k, v, masks, wq_w, wq_b, wk_w, wk_b, wv_w, wv_b, wo_w, wo_b):
    from concourse.bass_utils import run_bass_kernel_spmd

    q = np.asarray(q, np.float32)
    k = np.asarray(k, np.float32)
    v = np.asarray(v, np.float32)
    masks_np = np.asarray(masks)
    args = [np.asarray(a, np.float32) for a in
            (wq_w, wq_b, wk_w, wk_b, wv_w, wv_b, wo_w, wo_b)]
    wq_w, wq_b, wk_w, wk_b, wv_w, wv_b, wo_w, wo_b = args

    ndc = 5 if (np.any(wq_b) or np.any(wk_b) or np.any(wv_b)) else 4
    # key compaction: pad the max unmasked-key count to a 128 multiple
    max_nk = max(int(np.count_nonzero(masks_np[b])) for b in range(B))
    nkc = max(1, (max_nk + P - 1) // P)
    nc = _get_nc(ndc, nkc)

    in_maps = []
    for core in range(8):
        m = _prep_core(core, q, k, v, masks_np, wq_w, wq_b, wk_w, wk_b,
                       wv_w, wv_b, ndc, nkc)
        hg = core % 2
        rows = slice(hg * GD, (hg + 1) * GD)
        m["woT"] = np.ascontiguousarray(
            wo_w[:, rows].T.reshape(2, P, D).transpose(1, 0, 2))
        in_maps.append(m)

    res = run_bass_kernel_spmd(nc, in_maps, core_ids=list(range(8)),
                               trace=_RUN_OPTS.get("trace", False),
                               tmpdir=_RUN_OPTS.get("tmpdir"))
    _CACHE["last_result"] = res
    outs = res.results

    O = np.zeros((B, L, D), np.float32)
    for b in range(B):
        O[b] = (outs[2 * b]["o"].reshape(L, D)
                + outs[2 * b + 1]["o"].reshape(L, D))
    O += (wv_b @ wo_w.T + wo_b)[None, None, :] if ndc == 4 else wo_b[None, None, :]
    return O


# revision 15
# speedup vs baseline: 1.7337x; 1.0212x over previous
"""Multi-head attention (B=4, L=2048, D=512, H=8) on 8 Trainium2 NeuronCores.

Sharding: core = (batch b, head-group hg) -> each core handles 1 batch and 4
heads (tensor-parallel column-shard of Wq/Wk/Wv, row-shard of Wo). The two
head-group partial outputs per batch are summed on the host (the TP
all-reduce step of the gather).

Device dataflow (all f32, everything contraction-on-partitions, zero on-chip
transposes):
  - Host pre-transposes activations (qT/kT/vT: [512, 2048]) and weights.
  - Projections: qhT/khT = Wx^T.T @ xT -> [64*4, 2048] per-head-transposed
    layouts; vh = xT.T @ WvT -> [2048, 4*65] (keys on partitions) with a
    65th "ones" column per head.
  - Mask folding: vh rows of masked keys are zeroed (masked keys then
    contribute nothing to either the context numerator or - via the ones
    column - the softmax denominator). The ones column is overwritten with
    the 0/1 mask, so column 64 of the second matmul output IS the softmax
    denominator sum.
  - Scores are computed transposed: ST[k, q] = khT_h.T @ qhT_h, exp on
    ScalarE straight out of PSUM (no max-subtraction: scores ~ N(0,1), the
    shift is mathematically redundant), then ctxT[dk+1, q] accumulates
    vh_h.T @ exp(ST) over key chunks.
  - Normalization once on the small ctxT: r = 1/sums broadcast across
    partitions with a rank-1 PE matmul, one elementwise multiply.
  - Output projection O = ctxT.T @ WoT per 128-row chunk, DMA to DRAM.
"""
import os
import sys

import numpy as np

for _p in ("/opt/trn_rl_repo", "/root/.axon_site/_ro/trn_rl_repo"):
    if os.path.isdir(_p) and _p not in sys.path:
        sys.path.insert(0, _p)

B, L, D, H = 4, 2048, 512, 8
DK = D // H          # 64
HPG = 4              # heads per group
GD = HPG * DK        # 256
P = 128
NKC = L // P         # 16 key chunks
NLB = L // 512       # 4 l-blocks of 512
NLC = L // P         # 16 l chunks

_CACHE: dict = {}
# test harness hooks: set _RUN_OPTS["trace"]=True to request an NTFF profile;
# the last BassKernelResults lands in _CACHE["last_result"].
_RUN_OPTS: dict = {"trace": False}


def _build_nc(ndc: int, nkc: int):
    """Build + compile the Bass program.

    ndc: 4 normally, 5 when q/k/v biases are nonzero (extra contraction chunk
    carrying a ones row x bias row).
    nkc: number of 128-key chunks after host-side compaction of masked-out
    keys (masked keys contribute exactly nothing - their vh rows and mask
    column are zero - so only unmasked keys are shipped/computed; padding
    keys have zero vh/mask rows and zero kh, so exp(0)=1 times 0 = 0).
    """
    from contextlib import ExitStack

    import concourse.bacc as bacc
    import concourse.tile as tile
    from concourse import mybir

    f32 = mybir.dt.float32
    f32r = mybir.dt.float32r
    EXP = mybir.ActivationFunctionType.Exp
    MULT = mybir.AluOpType.mult

    nc = bacc.Bacc("TRN2", target_bir_lowering=False, debug=False, num_devices=8)

    NKP = nkc * P
    NKB = (NKP + 511) // 512  # key-axis 512-blocks for the K projection
    qT = nc.dram_tensor("qT", [ndc, P, L], f32r, kind="ExternalInput").ap()
    kT = nc.dram_tensor("kT", [ndc, P, NKP], f32r, kind="ExternalInput").ap()
    vT = nc.dram_tensor("vT", [ndc, P, NKP], f32r, kind="ExternalInput").ap()
    wqT = nc.dram_tensor("wqT", [P, ndc, GD], f32r, kind="ExternalInput").ap()
    wkT = nc.dram_tensor("wkT", [P, ndc, GD], f32r, kind="ExternalInput").ap()
    wvT = nc.dram_tensor("wvT", [P, ndc, HPG * 65], f32r, kind="ExternalInput").ap()
    woT = nc.dram_tensor("woT", [P, 2, D], f32r, kind="ExternalInput").ap()
    maskp = nc.dram_tensor("maskp", [P, nkc], f32, kind="ExternalInput").ap()
    o = nc.dram_tensor("o", [NLC, P, D], f32, kind="ExternalOutput").ap()

    with ExitStack() as ctx:
        tc = ctx.enter_context(tile.TileContext(nc))
        const = ctx.enter_context(tc.tile_pool(name="const", bufs=1))
        persist = ctx.enter_context(tc.tile_pool(name="persist", bufs=1))

        # constants / weights
        wq_sb = const.tile([P, ndc, GD], f32r)
        wk_sb = const.tile([P, ndc, GD], f32r)
        wv_sb = const.tile([P, ndc, HPG * 65], f32r)
        wo_sb = const.tile([P, 2, D], f32r)
        maskp_sb = const.tile([P, nkc], f32)
        ones_sb = const.tile([1, DK], f32)
        dummy_sb = const.tile([1, 8], f32)
        nc.sync.dma_start(wq_sb, wqT)
        nc.sync.dma_start(wk_sb, wkT)
        nc.sync.dma_start(wv_sb, wvT)
        nc.sync.dma_start(wo_sb, woT)
        nc.sync.dma_start(maskp_sb, maskp)
        nc.vector.memset(ones_sb, 1.0)
        # preload the exp table set early (overlaps the projection phase)
        nc.vector.memset(dummy_sb, 0.0)
        nc.scalar.activation(dummy_sb, dummy_sb, EXP)

        # persistent activations
        qh_sb = [persist.tile([P, L], f32r, name=f"qh{i}") for i in range(2)]
        kh_sb = [persist.tile([P, NKP], f32r, name=f"kh{i}") for i in range(2)]
        vh_sb = persist.tile([P, nkc, HPG, 65], f32r, name="vh")
        ctx_sb = [persist.tile([P, L], f32r, name=f"ctx{i}") for i in range(2)]

        # ---------------- projections ----------------
        with tc.tile_pool(name="xT", bufs=ndc) as xpool, \
             tc.tile_pool(name="ppsum", bufs=4, space="PSUM") as ppsum:
            for xap, w_sb, dst, xw in ((kT, wk_sb, kh_sb, NKP),
                                       (qT, wq_sb, qh_sb, L)):
                xt = [xpool.tile([P, L], f32r, tag="xT", name=f"xt{dc}") for dc in range(ndc)]
                for dc in range(ndc):
                    nc.sync.dma_start(xt[dc][:, 0:xw], xap[dc])
                for hp in range(2):
                    for lb in range((xw + 511) // 512):
                        nb = min(512, xw - lb * 512)
                        ps = ppsum.tile([P, 512], f32, tag="pp", name="ps_qk")
                        for dc in range(ndc):
                            nc.tensor.matmul(
                                ps[:, 0:nb],
                                lhsT=w_sb[:, dc, hp * P:(hp + 1) * P],
                                rhs=xt[dc][:, lb * 512:lb * 512 + nb],
                                start=(dc == 0),
                                stop=(dc == ndc - 1),
                            )
                        nc.vector.tensor_copy(dst[hp][:, lb * 512:lb * 512 + nb],
                                              ps[:, 0:nb])
            # V projection: vh[l, :] with mask fold
            xt = [xpool.tile([P, L], f32r, tag="xT", name=f"xt{dc}") for dc in range(ndc)]
            for dc in range(ndc):
                nc.sync.dma_start(xt[dc][:, 0:NKP], vT[dc])
            for lc in range(nkc):
                ps = ppsum.tile([P, HPG * 65], f32, tag="pp", name="ps_v")
                for dc in range(ndc):
                    nc.tensor.matmul(
                        ps,
                        lhsT=xt[dc][:, lc * P:(lc + 1) * P],
                        rhs=wv_sb[:, dc, :],
                        start=(dc == 0),
                        stop=(dc == ndc - 1),
                    )
                nc.vector.tensor_scalar_mul(
                    vh_sb[:, lc, :, :], ps.rearrange("p (h d) -> p h d", h=HPG),
                    maskp_sb[:, lc:lc + 1],
                )
                # ones-column -> 0/1 mask column (weights there are zero)
                nc.vector.tensor_copy(
                    vh_sb[:, lc, :, DK:DK + 1],
                    maskp_sb[:, lc:lc + 1, None].to_broadcast((P, HPG, 1)),
                )

        # ---------------- attention + output projection ----------------
        with tc.tile_pool(name="spsum", bufs=2, space="PSUM") as s_pool, \
             tc.tile_pool(name="cpsum", bufs=2, space="PSUM") as ctx_pool, \
             tc.tile_pool(name="pt", bufs=6) as pt_pool, \
             tc.tile_pool(name="nrm", bufs=2) as nrm_pool, \
             tc.tile_pool(name="osb", bufs=3) as o_pool:
            for q2 in range(2):           # q halves of 1024
                q0 = q2 * 1024
                for h in range(HPG):
                    hp, hb = h // 2, (h % 2) * DK
                    ctx_ps = ctx_pool.tile([65, 1024], f32, tag="ctx", name="ctx_ps")

                    def emit_ctx(pv, _ctx=ctx_ps, _h=h):
                        pt_prev, kcp = pv
                        vlhsT = vh_sb[:, kcp, _h, :]
                        for j in range(2):
                            nc.tensor.matmul(
                                _ctx[:, j * 512:(j + 1) * 512],
                                lhsT=vlhsT,
                                rhs=pt_prev[:, j * 512:(j + 1) * 512],
                                start=(kcp == 0), stop=(kcp == nkc - 1),
                            )

                    prev = None
                    for kc in range(nkc):
                        s_ps = s_pool.tile([P, 1024], f32, tag="s", name="s_ps")
                        lhsT = kh_sb[hp][hb:hb + DK, kc * P:(kc + 1) * P]
                        for j in range(2):
                            nc.tensor.matmul(
                                s_ps[:, j * 512:(j + 1) * 512],
                                lhsT=lhsT,
                                rhs=qh_sb[hp][hb:hb + DK, q0 + j * 512:q0 + (j + 1) * 512],
                                start=True, stop=True,
                            )
                        # software pipeline: the previous chunk's second matmul
                        # goes ahead of this chunk's exp in the PE stream, so
                        # the array never waits on ScalarE
                        if prev is not None:
                            emit_ctx(prev)
                        pt = pt_pool.tile([P, 1024], f32r, tag="pt", name="pt")
                        nc.scalar.activation(pt, s_ps, EXP)
                        prev = (pt, kc)
                    emit_ctx(prev)
                    # normalize: ctx_sb = ctx_ps[0:64] * (1/sums) broadcast
                    srow = nrm_pool.tile([1, 1024], f32, tag="srow", name="srow")
                    nc.vector.tensor_copy(srow, ctx_ps[64:65, :])
                    rrow = nrm_pool.tile([1, 1024], f32, tag="rrow", name="rrow")
                    nc.vector.reciprocal_approx_fast(rrow, srow)
                    bc_sb = nrm_pool.tile([DK, 1024], f32, tag="bc", name="bc_sb")
                    nc.gpsimd.partition_broadcast(bc_sb, rrow)
                    nc.vector.tensor_tensor(
                        ctx_sb[hp][hb:hb + DK, q0:q0 + 1024],
                        ctx_ps[0:DK, :],
                        bc_sb,
                        MULT,
                    )
                # output projection for this q half (8 l-chunks of 128)
                for lc in range(q2 * 8, q2 * 8 + 8):
                    ps = s_pool.tile([P, 1024], f32, tag="s", name="ps_o")
                    for c2 in range(2):
                        nc.tensor.matmul(
                            ps[:, 0:D],
                            lhsT=ctx_sb[c2][:, lc * P:(lc + 1) * P],
                            rhs=wo_sb[:, c2, :],
                            start=(c2 == 0), stop=(c2 == 1),
                        )
                    ot = o_pool.tile([P, D], f32, tag="o", name="ot")
                    nc.scalar.copy(ot, ps[:, 0:D])
                    nc.sync.dma_start(o[lc], ot)

    nc.compile()
    return nc


def _get_nc(ndc: int, nkc: int):
    key = ("nc", ndc, nkc)
    if key not in _CACHE:
        _CACHE[key] = _build_nc(ndc, nkc)
    return _CACHE[key]


def _prep_core(core, q, k, v, masks, wq_w, wq_b, wk_w, wk_b, wv_w, wv_b, ndc,
               nkc):
    b, hg = core // 2, core % 2
    rows = slice(hg * GD, (hg + 1) * GD)
    scale = np.float32(1.0 / np.sqrt(DK))
    NKP = nkc * P
    idx = np.nonzero(masks[b])[0]          # unmasked key positions

    def xt_chunks(x, compact):
        w = NKP if compact else L
        xt = np.zeros((ndc, P, w), np.float32)
        xs = x[idx] if compact else x      # [nk or L, 512]
        xt[:4, :, :xs.shape[0]] = np.ascontiguousarray(xs.T).reshape(4, P, -1)
        if ndc == 5:
            xt[4, 0, :xs.shape[0]] = 1.0   # ones row for the bias chunk
        return xt

    def w_chunks(wT, bias, width):
        w = np.zeros((ndc * P, width), np.float32)
        w[:D] = wT
        if ndc == 5:
            w[D] = bias
        return np.ascontiguousarray(w.reshape(ndc, P, width).transpose(1, 0, 2))

    wqT = (wq_w[rows, :].T * scale).astype(np.float32)          # [512, 256]
    wkT = wk_w[rows, :].T.astype(np.float32)
    wvT = np.zeros((D, HPG * 65), np.float32)
    wvb = np.zeros((HPG * 65,), np.float32)
    wvg = wv_w[rows, :]
    for hh in range(HPG):
        wvT[:, hh * 65:hh * 65 + DK] = wvg[hh * DK:(hh + 1) * DK].T
        wvb[hh * 65:hh * 65 + DK] = wv_b[rows][hh * DK:(hh + 1) * DK]
    maskc = np.zeros((NKP,), np.float32)
    maskc[:len(idx)] = 1.0
    return {
        "qT": xt_chunks(q[b], False),
        "kT": xt_chunks(k[b], True),
        "vT": xt_chunks(v[b], True),
        "wqT": w_chunks(wqT, wq_b[rows] * scale, GD),
        "wkT": w_chunks(wkT, wk_b[rows], GD),
        "wvT": w_chunks(wvT, wvb, HPG * 65),
        "maskp": np.ascontiguousarray(
            maskc.reshape(nkc, P).T.astype(np.float32)),
    }


def kernel(q, k, v, masks, wq_w, wq_b, wk_w, wk_b, wv_w, wv_b, wo_w, wo_b):
    from concourse.bass_utils import run_bass_kernel_spmd

    q = np.asarray(q, np.float32)
    k = np.asarray(k, np.float32)
    v = np.asarray(v, np.float32)
    masks_np = np.asarray(masks)
    args = [np.asarray(a, np.float32) for a in
            (wq_w, wq_b, wk_w, wk_b, wv_w, wv_b, wo_w, wo_b)]
    wq_w, wq_b, wk_w, wk_b, wv_w, wv_b, wo_w, wo_b = args

    ndc = 5 if (np.any(wq_b) or np.any(wk_b) or np.any(wv_b)) else 4
    # key compaction: pad the max unmasked-key count to a 128 multiple
    max_nk = max(int(np.count_nonzero(masks_np[b])) for b in range(B))
    nkc = max(1, (max_nk + P - 1) // P)
    nc = _get_nc(ndc, nkc)

    in_maps = []
    for core in range(8):
        m = _prep_core(core, q, k, v, masks_np, wq_w, wq_b, wk_w, wk_b,
                       wv_w, wv_b, ndc, nkc)
        hg = core % 2
        rows = slice(hg * GD, (hg + 1) * GD)
        m["woT"] = np.ascontiguousarray(
            wo_w[:, rows].T.reshape(2, P, D).transpose(1, 0, 2))
        in_maps.append(m)

    res = run_bass_kernel_spmd(nc, in_maps, core_ids=list(range(8)),
                               trace=_RUN_OPTS.get("trace", False),
                               tmpdir=_RUN_OPTS.get("tmpdir"))
    _CACHE["last_result"] = res
    outs = res.results

    O = np.zeros((B, L, D), np.float32)
    for b in range(B):
        O[b] = (outs[2 * b]["o"].reshape(L, D)
                + outs[2 * b + 1]["o"].reshape(L, D))
    O += (wv_b @ wo_w.T + wo_b)[None, None, :] if ndc == 4 else wo_b[None, None, :]
    return O
